# revision 55
# baseline (speedup 1.0000x reference)
"""Cross-attention kernel for Trainium2 (8 NeuronCores, batch-parallel).

Reference computation (per batch element b):
    q = x @ Wq + bq            # [T, E]
    k = y @ Wk                 # [S, E]  (k bias cancels in softmax)
    v = y @ Wv                 # [S, E]  (v bias folds into bo on host)
    per head h (D=80): scores = q_h @ k_h.T / sqrt(D); A = softmax(scores)
    attn = concat_h(A @ v_h)   # [T, E]
    out = attn @ Wo + bo'      # [T, E], bo' = bo + bv @ Wo

Sharding: batch (8) across the 8 cores, one batch element per core.

Feature-major on-chip layout (x and the output transposed on the host so
every DMA is a contiguous row load/store). x and all weights are loaded
as bf16 (halves HBM load traffic; PE rate for bf16 moving operands is
identical to f32r). PSUM accumulation stays f32; the output is f32.

Per-chunk pipeline (512 tokens), engines balanced so PE is the only
near-saturated engine (gpsimd/Pool cannot touch PSUM on real HW, so all
psum->sbuf moves sit on DVE/Act):
    PE  : scores(k) -> Qproj(k+1) -> AV(k) -> Oproj(k)
    Act : q psum->sbuf copy (+bq via activation bias), exp(k), half the
          out biases, wq/wv setup DMA
    Pool: partition_all_reduce (sumexp), A *= 1/sumexp, setup DMA
          (bqt/kmask/yt/wk blocks), last-chunk store fan-out
    DVE : reciprocal, attn psum->sbuf copies, half the out biases
    SP  : x chunk loads, wv/wo loads, output stores

The Qproj(k+1) placement between scores(k) and AV(k) keeps the in-order
PE busy while the softmax chain (Act->Pool->DVE->Pool) for chunk k
drains. A zero-tile matmul warmup burns the initial DMA-latency window
so the PE p-state ramp (0.65->1.2->2.4GHz over ~3us of continuous work)
completes before Qproj(0) issues. wq/wk are host-packed into
per-out-tile [128, contraction] blocks so the first projections start
as soon as the first block lands.
"""

import numpy as np
import ml_dtypes

import concourse.bass as bass
import concourse.bass_isa as bass_isa
import concourse.mybir as mybir
import concourse.tile as tile
from concourse import bacc
from concourse.bass_utils import run_bass_kernel_spmd

F32 = mybir.dt.float32
F32R = mybir.dt.float32r
BF16 = mybir.dt.bfloat16
AF = mybir.ActivationFunctionType

B, T, E, CR, H, D, S = 8, 4096, 640, 768, 8, 80, 77
TC = 512
# (t0, width) chunks; the last 512 is split so the drain phase (softmax ->
# AV -> O -> store with no next-chunk Qproj to overlap) is half as deep.
CHUNKS = [(512 * i, 512) for i in range(7)] + [(3584, 256), (3840, 256)]
NT = len(CHUNKS)
EJ = E // 128            # 5 e-tiles
CJ = CR // 128            # 6 cross-dim tiles
SCALE = float(1.0 / np.sqrt(D))


def _frags():
    fr = []
    for h in range(H):
        e0, e1 = D * h, D * h + D
        for j in range(e0 // 128, (e1 - 1) // 128 + 1):
            p0, p1 = max(0, e0 - 128 * j), min(128, e1 - 128 * j)
            fr.append((h, j, p0, p1))
    return fr


FRAGS = _frags()
NF = len(FRAGS)


def _emit(nc, tc, dr):
    import contextlib

    ctx = contextlib.ExitStack()
    with ctx:
        cpool = ctx.enter_context(tc.tile_pool(name="const", bufs=1))
        pq = ctx.enter_context(tc.tile_pool(name="pq", bufs=2, space="PSUM"))
        psc = ctx.enter_context(tc.tile_pool(name="psc", bufs=2, space="PSUM"))
        pav = ctx.enter_context(tc.tile_pool(name="pav", bufs=2, space="PSUM"))
        pop = ctx.enter_context(tc.tile_pool(name="pop", bufs=2, space="PSUM"))
        xpool = ctx.enter_context(tc.tile_pool(name="xpool", bufs=2))
        qpool = ctx.enter_context(tc.tile_pool(name="qpool", bufs=2))
        apool = ctx.enter_context(tc.tile_pool(name="apool", bufs=2))
        arpool = ctx.enter_context(tc.tile_pool(name="arpool", bufs=3))
        atpool = ctx.enter_context(tc.tile_pool(name="atpool", bufs=2))
        opool = ctx.enter_context(tc.tile_pool(name="opool", bufs=4))

        def load(name, shape, src, dt=F32, eng=None):
            t = cpool.tile(shape, dt, tag=name, name=name)
            (eng or nc.sync).dma_start(t[:], src)
            return t

        def loadw(name, src2, nblk, cols, dt=BF16, eng=None):
            t = cpool.tile([128, nblk, cols], dt, tag=name, name=name)
            (eng or nc.sync).dma_start(
                t[:], src2.rearrange("(b p) c -> p b c", p=128))
            return t

        # ---- load order == DMA order on SP; it gates PE start times.
        # x chunk 0 + per-out-tile wq blocks first so Qproj(0) starts the
        # moment x0 and the first block land; K inputs next (scores(0)),
        # then x1 (Qproj(1)), wv (AV(0)), wo (Oproj(0)).
        def load_x(t0, tw, qtag, engs=None):
            xp = []
            for c in range(EJ):
                xt = xpool.tile([128, TC], BF16, tag=f"xp{c}",
                                name=f"x_{qtag}_{c}")
                eng = engs[c] if engs else nc.sync
                eng.dma_start(xt[0:128, 0:tw],
                              dr["xt"][128 * c:128 * (c + 1), t0:t0 + tw])
                xp.append(xt)
            return xp

        # PE p-state warmup: the clock needs ~3us of continuous work to
        # reach 2.4GHz. Burn the initial DMA-latency window (PE would idle
        # anyway) on dummy matmuls over a zeroed tile so Qproj(0) starts
        # at full clock.
        wz = cpool.tile([128, TC], BF16, tag="wz", name="wz")
        nc.gpsimd.memset(wz[:], 0)
        wp = pq.tile([128, TC], F32, tag="qp", name="warm")
        for i in range(4):
            nc.tensor.matmul(wp[:], wz[:, 0:128], wz[:],
                             start=(i == 0), stop=(i == 3))

        # Setup loads fan out over the three DMA-capable engines so no
        # single sequencer serializes the ~8us of weight/x traffic:
        #   SP: x chunks (and all steady-state loads/stores)
        #   Act: wq blocks, then wv
        #   Pool: yt, wk blocks, masks, then wo
        wq_sb = [load(f"wq{j}", [128, E],
                      dr["wq"][128 * j:128 * (j + 1), :], BF16,
                      eng=nc.scalar)
                 for j in range(EJ)]
        # chunk 0's last two x tiles ride Pool's queue so Qproj(0) isn't
        # gated by SP's serial transfer of all five
        xp0 = load_x(0, CHUNKS[0][1], "q0",
                     engs=[nc.sync, nc.sync, nc.sync, nc.gpsimd, nc.gpsimd])
        bqt = load("bqt", [128, EJ], dr["bqt"][:], eng=nc.gpsimd)
        kmask = load("kmask", [128, NF], dr["kmask"][:], eng=nc.gpsimd)
        ytp_t = loadw("yt", dr["yt"][:], CJ, S, eng=nc.gpsimd)
        wkb = [load(f"wkb{j}", [128, CR],
                    dr["wk"][128 * j:128 * (j + 1), :], BF16,
                    eng=nc.gpsimd)
               for j in range(EJ)]
        xp1 = load_x(TC, TC, "q1")
        wv_t = loadw("wv", dr["wv"][:], CJ, E)
        vmask = load("vmask", [S, 128 * NF], dr["vmask"][:])
        wo_t = loadw("wo", dr["wo"][:], EJ, E)
        bot = load("bot", [128, EJ], dr["bot"][:])
        wv_sb = [wv_t[:, c, :] for c in range(CJ)]
        wo_sb = [wo_t[:, c, :] for c in range(EJ)]
        ytp = [ytp_t[:, c, :] for c in range(CJ)]

        def emit_qproj(xp, tw, qtag):
            qs = []
            for j in range(EJ):
                qp = pq.tile([128, TC], F32, tag="qp", name=f"qp{j}")
                for c in range(EJ):
                    nc.tensor.matmul(qp[0:128, 0:tw],
                                     wq_sb[j][:, 128 * c:128 * (c + 1)],
                                     xp[c][0:128, 0:tw],
                                     start=(c == 0), stop=(c == EJ - 1))
                q = qpool.tile([128, TC], BF16, tag=f"q{j}", name=f"{qtag}_{j}")
                with nc.allow_low_precision(reason="bf16 attention"):
                    nc.scalar.activation(q[0:128, 0:tw], qp[0:128, 0:tw],
                                         AF.Identity, bias=bqt[:, j:j + 1])
                qs.append(q)
            return qs

        qs_cur = emit_qproj(xp0, TC, "q0")

        # ---- K projection -> per-fragment zero-masked staging (bf16) ----
        kstage = [cpool.tile([128, S], BF16, tag=f"ks{fi}", name=f"ks{fi}")
                  for fi in range(NF)]
        for j in range(EJ):
            kp = pq.tile([128, S], F32, tag="qp")
            for c in range(CJ):
                nc.tensor.matmul(kp[:], wkb[j][:, 128 * c:128 * (c + 1)],
                                 ytp[c], start=(c == 0), stop=(c == CJ - 1))
            for fi, (h, jj, p0, p1) in enumerate(FRAGS):
                if jj != j:
                    continue
                with nc.allow_low_precision(reason="bf16 attention"):
                    nc.vector.tensor_scalar_mul(
                        kstage[fi][:], kp[:], kmask[:, fi:fi + 1])

        vb = [cpool.tile([S, 128], BF16, tag=f"vb{fi}", name=f"vb{fi}")
              for fi in range(NF)]

        def emit_vproj():
            # V projection -> per-fragment zero-masked vb tiles (bf16)
            for (n0, n1) in ((0, 512), (512, E)):
                vp = psc.tile([S, n1 - n0], F32, tag="sc")
                for c in range(CJ):
                    nc.tensor.matmul(vp[:], ytp[c], wv_sb[c][:, n0:n1],
                                     start=(c == 0), stop=(c == CJ - 1))
                for fi, (h, j, p0, p1) in enumerate(FRAGS):
                    c0 = 128 * j
                    if not (n0 <= c0 and c0 + 128 <= n1):
                        continue
                    with nc.allow_low_precision(reason="bf16 attention"):
                        nc.vector.tensor_mul(vb[fi][:],
                                             vp[:, c0 - n0:c0 - n0 + 128],
                                             vmask[:, 128 * fi:128 * (fi + 1)])

        # ---- main loop over token chunks (software-pipelined) ----
        xp_next = xp1
        for it in range(NT):
            t0, tw = CHUNKS[it]
            # prefetch x for chunk it+1 (SP); chunk 1 was hoisted above
            if it >= 1 and it + 1 < NT:
                nt0, ntw = CHUNKS[it + 1]
                xp_next = load_x(nt0, ntw, f"q{it + 1}")

            # scores -> exp -> sumexp(all-reduce) -> recip -> normalize
            aps = []
            for h in range(H):
                frs = [(fi, f) for fi, f in enumerate(FRAGS) if f[0] == h]
                sc = psc.tile([S, TC], F32, tag="sc")
                for i, (fi, (hh, j, p0, p1)) in enumerate(frs):
                    nc.tensor.matmul(sc[0:S, 0:tw], kstage[fi][:],
                                     qs_cur[j][0:128, 0:tw],
                                     start=(i == 0), stop=(i == len(frs) - 1))
                a = apool.tile([S, TC], BF16, tag=f"a{h}")
                with nc.allow_low_precision(reason="bf16 attention"):
                    nc.scalar.activation(a[0:S, 0:tw], sc[0:S, 0:tw],
                                         AF.Exp, scale=SCALE)
                # f32 sumexp so reciprocal_approx_fast (single custom DVE
                # op, ~5x cheaper than bit-exact reciprocal on HW) applies
                ar = arpool.tile([S, TC], F32, tag="ar")
                with nc.allow_low_precision(reason="bf16 attention"):
                    nc.gpsimd.partition_all_reduce(
                        ar[0:S, 0:tw], a[0:S, 0:tw], channels=S,
                        reduce_op=bass_isa.ReduceOp.add)
                    nc.vector.reciprocal_approx_fast(
                        out=ar[0:S, 0:tw], in_=ar[0:S, 0:tw])
                    nc.gpsimd.tensor_mul(a[0:S, 0:tw], a[0:S, 0:tw],
                                         ar[0:S, 0:tw])
                aps.append(a)

            # Qproj for the NEXT chunk fills PE while softmax drains
            if it + 1 < NT:
                qs_cur = emit_qproj(xp_next, CHUNKS[it + 1][1], f"q{it + 1}")
            if it == 0:
                emit_vproj()

            # AV: accumulate head fragments into attn' tiles (bf16)
            attn = []
            for j in range(EJ):
                av = pav.tile([128, TC], F32, tag="av")
                frs = [(fi, f) for fi, f in enumerate(FRAGS) if f[1] == j]
                for i, (fi, (h, jj, p0, p1)) in enumerate(frs):
                    nc.tensor.matmul(av[0:128, 0:tw], vb[fi][:],
                                     aps[h][0:S, 0:tw],
                                     start=(i == 0), stop=(i == len(frs) - 1))
                at = atpool.tile([128, TC], BF16, tag=f"at{j}")
                with nc.allow_low_precision(reason="bf16 attention"):
                    nc.vector.tensor_copy(at[0:128, 0:tw], av[0:128, 0:tw])
                attn.append(at)

            # O projection + bias (f32 out), store via SP
            for p in range(EJ):
                op = pop.tile([128, TC], F32, tag="op")
                for j in range(EJ):
                    nc.tensor.matmul(op[0:128, 0:tw],
                                     wo_sb[j][:, 128 * p:128 * (p + 1)],
                                     attn[j][0:128, 0:tw],
                                     start=(j == 0), stop=(j == EJ - 1))
                ob = opool.tile([128, TC], F32, tag="ob")
                if p % 2 == 0 and it != NT - 2:
                    nc.vector.tensor_scalar_add(ob[0:128, 0:tw],
                                                op[0:128, 0:tw],
                                                bot[:, p:p + 1])
                else:
                    nc.scalar.activation(ob[0:128, 0:tw], op[0:128, 0:tw],
                                         AF.Identity, bias=bot[:, p:p + 1])
                # drain stores across SP/Act/Pool on the last chunk so the
                # final writeback isn't serialized on one engine
                if it < NT - 2:
                    seng = nc.sync
                elif it == NT - 2:
                    seng = (nc.sync, nc.gpsimd)[p % 2]
                else:
                    seng = (nc.sync, nc.gpsimd, nc.sync, nc.gpsimd,
                            nc.scalar)[p]
                seng.dma_start(dr["ot"][128 * p:128 * (p + 1), t0:t0 + tw],
                               ob[0:128, 0:tw])


def build_program():
    nc = bacc.Bacc("TRN2", target_bir_lowering=False, debug=False,
                   num_devices=B)
    dr = {}

    def din(name, shape, dt=F32):
        dr[name] = nc.dram_tensor(name, shape, dt, kind="ExternalInput")
        return dr[name]

    din("xt", [E, T], BF16)
    din("yt", [CR, S], BF16)
    din("wq", [E, E], BF16)
    din("wk", [E, CR], BF16)
    din("wv", [CR, E], BF16)
    din("wo", [E, E], BF16)
    din("bqt", [128, EJ])
    din("bot", [128, EJ])
    din("kmask", [128, NF])
    din("vmask", [S, 128 * NF])
    dr["ot"] = nc.dram_tensor("ot", [E, T], F32, kind="ExternalOutput")

    with tile.TileContext(nc) as tc:
        _emit(nc, tc, {k: v[:] for k, v in dr.items()})
    nc.compile()
    return nc


def make_in_maps(x, y, Wq, bq, Wk, bk, Wv, bv, Wo, bo):
    bf = lambda a: np.ascontiguousarray(
        np.asarray(a, dtype=np.float32).astype(ml_dtypes.bfloat16))
    f = lambda a: np.ascontiguousarray(np.asarray(a, dtype=np.float32))
    kmask = np.zeros((128, NF), np.float32)
    vmask = np.zeros((S, 128 * NF), np.float32)
    for fi, (h, j, p0, p1) in enumerate(FRAGS):
        kmask[p0:p1, fi] = 1.0
        vmask[:, 128 * fi + p0:128 * fi + p1] = 1.0
    # exact folds: k bias cancels in softmax; v bias folds into bo.
    bo_eff = (np.asarray(bo, np.float32)
              + np.asarray(bv, np.float32) @ np.asarray(Wo, np.float32))
    # wq packed per out-col-block j: wqp[j, p, 128*b + c] = Wq[128*b + p,
    # 128*j + c] so Qproj out-tile j loads one contiguous [128, E] block.
    wqp = (np.asarray(Wq, np.float32)
           .reshape(EJ, 128, EJ, 128).transpose(2, 1, 0, 3)
           .reshape(E, E))
    # wk packed the same way: block j holds Wk[:, j-cols] as [128, CR]
    wkp = (np.asarray(Wk, np.float32)
           .reshape(CJ, 128, EJ, 128).transpose(2, 1, 0, 3)
           .reshape(E, CR))
    shared = dict(
        wq=bf(wqp), wk=bf(wkp), wv=bf(Wv), wo=bf(Wo),
        bqt=f(np.asarray(bq, np.float32).reshape(EJ, 128).T),
        bot=f(bo_eff.reshape(EJ, 128).T),
        kmask=kmask,
        vmask=vmask,
    )
    x = np.asarray(x, np.float32)
    y = np.asarray(y, np.float32)
    in_maps = []
    for b in range(B):
        m = dict(shared)
        m["xt"] = bf(x[b].T)
        m["yt"] = bf(y[b].T)
        in_maps.append(m)
    return in_maps


def assemble_output(results):
    return np.stack([results[b]["ot"].T for b in range(B)], axis=0)


_PROG = None


def _prog():
    global _PROG
    if _PROG is None:
        _PROG = build_program()
    return _PROG


def kernel(x, y, Wq, bq, Wk, bk, Wv, bv, Wo, bo):
    nc = _prog()
    in_maps = make_in_maps(x, y, Wq, bq, Wk, bk, Wv, bv, Wo, bo)
    res = run_bass_kernel_spmd(nc, in_maps, core_ids=list(range(B)))
    return assemble_output(res.results)


# revision 61
# speedup vs baseline: 1.0067x; 1.0067x over previous
"""Cross-attention kernel for Trainium2 (8 NeuronCores, batch-parallel).

Reference computation (per batch element b):
    q = x @ Wq + bq            # [T, E]
    k = y @ Wk                 # [S, E]  (k bias cancels in softmax)
    v = y @ Wv                 # [S, E]  (v bias folds into bo on host)
    per head h (D=80): scores = q_h @ k_h.T / sqrt(D); A = softmax(scores)
    attn = concat_h(A @ v_h)   # [T, E]
    out = attn @ Wo + bo'      # [T, E], bo' = bo + bv @ Wo

Sharding: batch (8) across the 8 cores, one batch element per core.

Feature-major on-chip layout (x and the output transposed on the host so
every DMA is a contiguous row load/store). x and all weights are loaded
as bf16 (halves HBM load traffic; PE rate for bf16 moving operands is
identical to f32r). PSUM accumulation stays f32; the output is f32.

Per-chunk pipeline (512 tokens), engines balanced so PE is the only
near-saturated engine (gpsimd/Pool cannot touch PSUM on real HW, so all
psum->sbuf moves sit on DVE/Act):
    PE  : scores(k) -> Qproj(k+1) -> AV(k) -> Oproj(k)
    Act : q psum->sbuf copy (+bq via activation bias), exp(k), half the
          out biases, wq/wv setup DMA
    Pool: partition_all_reduce (sumexp), A *= 1/sumexp, setup DMA
          (bqt/kmask/yt/wk blocks), last-chunk store fan-out
    DVE : reciprocal, attn psum->sbuf copies, half the out biases
    SP  : x chunk loads, wv/wo loads, output stores

The Qproj(k+1) placement between scores(k) and AV(k) keeps the in-order
PE busy while the softmax chain (Act->Pool->DVE->Pool) for chunk k
drains. A zero-tile matmul warmup burns the initial DMA-latency window
so the PE p-state ramp (0.65->1.2->2.4GHz over ~3us of continuous work)
completes before Qproj(0) issues. wq/wk are host-packed into
per-out-tile [128, contraction] blocks so the first projections start
as soon as the first block lands.
"""

import numpy as np
import ml_dtypes

import concourse.bass as bass
import concourse.bass_isa as bass_isa
import concourse.mybir as mybir
import concourse.tile as tile
from concourse import bacc
from concourse.bass_utils import run_bass_kernel_spmd

F32 = mybir.dt.float32
F32R = mybir.dt.float32r
BF16 = mybir.dt.bfloat16
AF = mybir.ActivationFunctionType

B, T, E, CR, H, D, S = 8, 4096, 640, 768, 8, 80, 77
TC = 512
# (t0, width) chunks; the last 512 is split so the drain phase (softmax ->
# AV -> O -> store with no next-chunk Qproj to overlap) is half as deep.
CHUNKS = [(512 * i, 512) for i in range(7)] + [(3584, 256), (3840, 256)]
NT = len(CHUNKS)
EJ = E // 128            # 5 e-tiles
CJ = CR // 128            # 6 cross-dim tiles
SCALE = float(1.0 / np.sqrt(D))


def _frags():
    fr = []
    for h in range(H):
        e0, e1 = D * h, D * h + D
        for j in range(e0 // 128, (e1 - 1) // 128 + 1):
            p0, p1 = max(0, e0 - 128 * j), min(128, e1 - 128 * j)
            fr.append((h, j, p0, p1))
    return fr


FRAGS = _frags()
NF = len(FRAGS)


def _emit(nc, tc, dr):
    import contextlib

    ctx = contextlib.ExitStack()
    with ctx:
        cpool = ctx.enter_context(tc.tile_pool(name="const", bufs=1))
        pq = ctx.enter_context(tc.tile_pool(name="pq", bufs=2, space="PSUM"))
        psc = ctx.enter_context(tc.tile_pool(name="psc", bufs=2, space="PSUM"))
        pav = ctx.enter_context(tc.tile_pool(name="pav", bufs=2, space="PSUM"))
        pop = ctx.enter_context(tc.tile_pool(name="pop", bufs=2, space="PSUM"))
        xpool = ctx.enter_context(tc.tile_pool(name="xpool", bufs=2))
        qpool = ctx.enter_context(tc.tile_pool(name="qpool", bufs=2))
        apool = ctx.enter_context(tc.tile_pool(name="apool", bufs=2))
        arpool = ctx.enter_context(tc.tile_pool(name="arpool", bufs=3))
        atpool = ctx.enter_context(tc.tile_pool(name="atpool", bufs=2))
        opool = ctx.enter_context(tc.tile_pool(name="opool", bufs=4))

        def load(name, shape, src, dt=F32, eng=None):
            t = cpool.tile(shape, dt, tag=name, name=name)
            (eng or nc.sync).dma_start(t[:], src)
            return t

        def loadw(name, src2, nblk, cols, dt=BF16, eng=None):
            t = cpool.tile([128, nblk, cols], dt, tag=name, name=name)
            (eng or nc.sync).dma_start(
                t[:], src2.rearrange("(b p) c -> p b c", p=128))
            return t

        # ---- load order == DMA order on SP; it gates PE start times.
        # x chunk 0 + per-out-tile wq blocks first so Qproj(0) starts the
        # moment x0 and the first block land; K inputs next (scores(0)),
        # then x1 (Qproj(1)), wv (AV(0)), wo (Oproj(0)).
        def load_x(t0, tw, qtag, engs=None):
            xp = []
            for c in range(EJ):
                xt = xpool.tile([128, TC], BF16, tag=f"xp{c}",
                                name=f"x_{qtag}_{c}")
                eng = engs[c] if engs else nc.sync
                eng.dma_start(xt[0:128, 0:tw],
                              dr["xt"][128 * c:128 * (c + 1), t0:t0 + tw])
                xp.append(xt)
            return xp

        # PE p-state warmup: the clock needs ~3us of continuous work to
        # reach 2.4GHz. Burn the initial DMA-latency window (PE would idle
        # anyway) on dummy matmuls over a zeroed tile so Qproj(0) starts
        # at full clock.
        wz = cpool.tile([128, TC], BF16, tag="wz", name="wz")
        nc.gpsimd.memset(wz[:], 0)
        wp = pav.tile([128, TC], F32, tag="av", name="warm")
        for i in range(4):
            nc.tensor.matmul(wp[:], wz[:, 0:128], wz[:],
                             start=(i == 0), stop=(i == 3))

        # Setup loads fan out over the three DMA-capable engines so no
        # single sequencer serializes the ~8us of weight/x traffic:
        #   SP: x chunks (and all steady-state loads/stores)
        #   Act: wq blocks, then wv
        #   Pool: yt, wk blocks, masks, then wo
        wq_sb = [load(f"wq{j}", [128, E],
                      dr["wq"][128 * j:128 * (j + 1), :], BF16,
                      eng=nc.scalar)
                 for j in range(EJ)]
        # chunk 0's last two x tiles ride Pool's queue so Qproj(0) isn't
        # gated by SP's serial transfer of all five
        xp0 = load_x(0, CHUNKS[0][1], "q0",
                     engs=[nc.sync, nc.sync, nc.sync, nc.gpsimd, nc.gpsimd])
        bqt = load("bqt", [128, EJ], dr["bqt"][:], eng=nc.gpsimd)
        kmask = load("kmask", [128, NF], dr["kmask"][:], eng=nc.gpsimd)
        ytp_t = loadw("yt", dr["yt"][:], CJ, S, eng=nc.gpsimd)
        wkb = [load(f"wkb{j}", [128, CR],
                    dr["wk"][128 * j:128 * (j + 1), :], BF16,
                    eng=nc.gpsimd)
               for j in range(EJ)]
        xp1 = load_x(TC, TC, "q1")
        wv_t = loadw("wv", dr["wv"][:], CJ, E)
        vmask = load("vmask", [S, 128 * NF], dr["vmask"][:])
        wo_t = loadw("wo", dr["wo"][:], EJ, E)
        bot = load("bot", [128, EJ], dr["bot"][:])
        wv_sb = [wv_t[:, c, :] for c in range(CJ)]
        wo_sb = [wo_t[:, c, :] for c in range(EJ)]
        ytp = [ytp_t[:, c, :] for c in range(CJ)]

        def emit_qproj(xp, tw, qtag):
            qs = []
            for j in range(EJ):
                qp = pq.tile([128, TC], F32, tag="qp", name=f"qp{j}")
                for c in range(EJ):
                    nc.tensor.matmul(qp[0:128, 0:tw],
                                     wq_sb[j][:, 128 * c:128 * (c + 1)],
                                     xp[c][0:128, 0:tw],
                                     start=(c == 0), stop=(c == EJ - 1))
                q = qpool.tile([128, TC], BF16, tag=f"q{j}", name=f"{qtag}_{j}")
                with nc.allow_low_precision(reason="bf16 attention"):
                    nc.scalar.activation(q[0:128, 0:tw], qp[0:128, 0:tw],
                                         AF.Identity, bias=bqt[:, j:j + 1])
                qs.append(q)
            return qs

        qs_cur = emit_qproj(xp0, TC, "q0")

        # ---- K projection -> per-fragment zero-masked staging (bf16) ----
        kstage = [cpool.tile([128, S], BF16, tag=f"ks{fi}", name=f"ks{fi}")
                  for fi in range(NF)]
        for j in range(EJ):
            kp = pop.tile([128, S], F32, tag="op")
            for c in range(CJ):
                nc.tensor.matmul(kp[:], wkb[j][:, 128 * c:128 * (c + 1)],
                                 ytp[c], start=(c == 0), stop=(c == CJ - 1))
            for fi, (h, jj, p0, p1) in enumerate(FRAGS):
                if jj != j:
                    continue
                with nc.allow_low_precision(reason="bf16 attention"):
                    if fi % 2 == 0:
                        nc.vector.tensor_scalar_mul(
                            kstage[fi][:], kp[:], kmask[:, fi:fi + 1])
                    else:
                        nc.scalar.activation(
                            kstage[fi][:], kp[:], AF.Identity,
                            scale=kmask[:, fi:fi + 1])

        vb = [cpool.tile([S, 128], BF16, tag=f"vb{fi}", name=f"vb{fi}")
              for fi in range(NF)]

        def emit_vproj():
            # V projection -> per-fragment zero-masked vb tiles (bf16)
            for (n0, n1) in ((0, 512), (512, E)):
                vp = pav.tile([S, n1 - n0], F32, tag="av")
                for c in range(CJ):
                    nc.tensor.matmul(vp[:], ytp[c], wv_sb[c][:, n0:n1],
                                     start=(c == 0), stop=(c == CJ - 1))
                for fi, (h, j, p0, p1) in enumerate(FRAGS):
                    c0 = 128 * j
                    if not (n0 <= c0 and c0 + 128 <= n1):
                        continue
                    with nc.allow_low_precision(reason="bf16 attention"):
                        nc.vector.tensor_mul(vb[fi][:],
                                             vp[:, c0 - n0:c0 - n0 + 128],
                                             vmask[:, 128 * fi:128 * (fi + 1)])

        # ---- main loop over token chunks (software-pipelined) ----
        xp_next = xp1
        for it in range(NT):
            t0, tw = CHUNKS[it]
            # prefetch x for chunk it+1 (SP); chunk 1 was hoisted above
            if it >= 1 and it + 1 < NT:
                nt0, ntw = CHUNKS[it + 1]
                xp_next = load_x(nt0, ntw, f"q{it + 1}")

            # scores -> exp -> sumexp(all-reduce) -> recip -> normalize
            aps = []
            for h in range(H):
                frs = [(fi, f) for fi, f in enumerate(FRAGS) if f[0] == h]
                sc = psc.tile([S, TC], F32, tag="sc")
                for i, (fi, (hh, j, p0, p1)) in enumerate(frs):
                    nc.tensor.matmul(sc[0:S, 0:tw], kstage[fi][:],
                                     qs_cur[j][0:128, 0:tw],
                                     start=(i == 0), stop=(i == len(frs) - 1))
                a = apool.tile([S, TC], BF16, tag=f"a{h}")
                with nc.allow_low_precision(reason="bf16 attention"):
                    nc.scalar.activation(a[0:S, 0:tw], sc[0:S, 0:tw],
                                         AF.Exp, scale=SCALE)
                # f32 sumexp so reciprocal_approx_fast (single custom DVE
                # op, ~5x cheaper than bit-exact reciprocal on HW) applies
                ar = arpool.tile([S, TC], F32, tag="ar")
                with nc.allow_low_precision(reason="bf16 attention"):
                    nc.gpsimd.partition_all_reduce(
                        ar[0:S, 0:tw], a[0:S, 0:tw], channels=S,
                        reduce_op=bass_isa.ReduceOp.add)
                    nc.vector.reciprocal_approx_fast(
                        out=ar[0:S, 0:tw], in_=ar[0:S, 0:tw])
                    nc.gpsimd.tensor_mul(a[0:S, 0:tw], a[0:S, 0:tw],
                                         ar[0:S, 0:tw])
                aps.append(a)

            # Qproj for the NEXT chunk fills PE while softmax drains
            if it + 1 < NT:
                qs_cur = emit_qproj(xp_next, CHUNKS[it + 1][1], f"q{it + 1}")
            if it == 0:
                emit_vproj()

            # AV: accumulate head fragments into attn' tiles (bf16)
            attn = []
            for j in range(EJ):
                av = pav.tile([128, TC], F32, tag="av")
                frs = [(fi, f) for fi, f in enumerate(FRAGS) if f[1] == j]
                for i, (fi, (h, jj, p0, p1)) in enumerate(frs):
                    nc.tensor.matmul(av[0:128, 0:tw], vb[fi][:],
                                     aps[h][0:S, 0:tw],
                                     start=(i == 0), stop=(i == len(frs) - 1))
                at = atpool.tile([128, TC], BF16, tag=f"at{j}")
                with nc.allow_low_precision(reason="bf16 attention"):
                    nc.vector.tensor_copy(at[0:128, 0:tw], av[0:128, 0:tw])
                attn.append(at)

            # O projection + bias (f32 out), store via SP
            for p in range(EJ):
                op = pop.tile([128, TC], F32, tag="op")
                for j in range(EJ):
                    nc.tensor.matmul(op[0:128, 0:tw],
                                     wo_sb[j][:, 128 * p:128 * (p + 1)],
                                     attn[j][0:128, 0:tw],
                                     start=(j == 0), stop=(j == EJ - 1))
                ob = opool.tile([128, TC], F32, tag="ob")
                if p % 2 == 0 and it != NT - 2:
                    nc.vector.tensor_scalar_add(ob[0:128, 0:tw],
                                                op[0:128, 0:tw],
                                                bot[:, p:p + 1])
                else:
                    nc.scalar.activation(ob[0:128, 0:tw], op[0:128, 0:tw],
                                         AF.Identity, bias=bot[:, p:p + 1])
                # drain stores across SP/Act/Pool on the last chunk so the
                # final writeback isn't serialized on one engine
                if it < NT - 2:
                    seng = nc.sync
                elif it == NT - 2:
                    seng = (nc.sync, nc.gpsimd)[p % 2]
                else:
                    seng = (nc.sync, nc.gpsimd, nc.sync, nc.gpsimd,
                            nc.scalar)[p]
                seng.dma_start(dr["ot"][128 * p:128 * (p + 1), t0:t0 + tw],
                               ob[0:128, 0:tw])


def build_program():
    nc = bacc.Bacc("TRN2", target_bir_lowering=False, debug=False,
                   num_devices=B)
    dr = {}

    def din(name, shape, dt=F32):
        dr[name] = nc.dram_tensor(name, shape, dt, kind="ExternalInput")
        return dr[name]

    din("xt", [E, T], BF16)
    din("yt", [CR, S], BF16)
    din("wq", [E, E], BF16)
    din("wk", [E, CR], BF16)
    din("wv", [CR, E], BF16)
    din("wo", [E, E], BF16)
    din("bqt", [128, EJ])
    din("bot", [128, EJ])
    din("kmask", [128, NF])
    din("vmask", [S, 128 * NF])
    dr["ot"] = nc.dram_tensor("ot", [E, T], F32, kind="ExternalOutput")

    with tile.TileContext(nc) as tc:
        _emit(nc, tc, {k: v[:] for k, v in dr.items()})
    nc.compile()
    return nc


def make_in_maps(x, y, Wq, bq, Wk, bk, Wv, bv, Wo, bo):
    bf = lambda a: np.ascontiguousarray(
        np.asarray(a, dtype=np.float32).astype(ml_dtypes.bfloat16))
    f = lambda a: np.ascontiguousarray(np.asarray(a, dtype=np.float32))
    kmask = np.zeros((128, NF), np.float32)
    vmask = np.zeros((S, 128 * NF), np.float32)
    for fi, (h, j, p0, p1) in enumerate(FRAGS):
        kmask[p0:p1, fi] = 1.0
        vmask[:, 128 * fi + p0:128 * fi + p1] = 1.0
    # exact folds: k bias cancels in softmax; v bias folds into bo.
    bo_eff = (np.asarray(bo, np.float32)
              + np.asarray(bv, np.float32) @ np.asarray(Wo, np.float32))
    # wq packed per out-col-block j: wqp[j, p, 128*b + c] = Wq[128*b + p,
    # 128*j + c] so Qproj out-tile j loads one contiguous [128, E] block.
    wqp = (np.asarray(Wq, np.float32)
           .reshape(EJ, 128, EJ, 128).transpose(2, 1, 0, 3)
           .reshape(E, E))
    # wk packed the same way: block j holds Wk[:, j-cols] as [128, CR]
    wkp = (np.asarray(Wk, np.float32)
           .reshape(CJ, 128, EJ, 128).transpose(2, 1, 0, 3)
           .reshape(E, CR))
    shared = dict(
        wq=bf(wqp), wk=bf(wkp), wv=bf(Wv), wo=bf(Wo),
        bqt=f(np.asarray(bq, np.float32).reshape(EJ, 128).T),
        bot=f(bo_eff.reshape(EJ, 128).T),
        kmask=kmask,
        vmask=vmask,
    )
    x = np.asarray(x, np.float32)
    y = np.asarray(y, np.float32)
    in_maps = []
    for b in range(B):
        m = dict(shared)
        m["xt"] = bf(x[b].T)
        m["yt"] = bf(y[b].T)
        in_maps.append(m)
    return in_maps


def assemble_output(results):
    return np.stack([results[b]["ot"].T for b in range(B)], axis=0)


_PROG = None


def _prog():
    global _PROG
    if _PROG is None:
        _PROG = build_program()
    return _PROG


def kernel(x, y, Wq, bq, Wk, bk, Wv, bv, Wo, bo):
    nc = _prog()
    in_maps = make_in_maps(x, y, Wq, bq, Wk, bk, Wv, bv, Wo, bo)
    res = run_bass_kernel_spmd(nc, in_maps, core_ids=list(range(B)))
    return assemble_output(res.results)


# revision 69
# speedup vs baseline: 1.0074x; 1.0008x over previous
"""Cross-attention kernel for Trainium2 (8 NeuronCores, batch-parallel).

Reference computation (per batch element b):
    q = x @ Wq + bq            # [T, E]
    k = y @ Wk                 # [S, E]  (k bias cancels in softmax)
    v = y @ Wv                 # [S, E]  (v bias folds into bo on host)
    per head h (D=80): scores = q_h @ k_h.T / sqrt(D); A = softmax(scores)
    attn = concat_h(A @ v_h)   # [T, E]
    out = attn @ Wo + bo'      # [T, E], bo' = bo + bv @ Wo

Sharding: batch (8) across the 8 cores, one batch element per core.

Feature-major on-chip layout (x and the output transposed on the host so
every DMA is a contiguous row load/store). x and all weights are loaded
as bf16 (halves HBM load traffic; PE rate for bf16 moving operands is
identical to f32r). PSUM accumulation stays f32; the output is f32.

Per-chunk pipeline (512 tokens), engines balanced so PE is the only
near-saturated engine (gpsimd/Pool cannot touch PSUM on real HW, so all
psum->sbuf moves sit on DVE/Act):
    PE  : scores(k) -> Qproj(k+1) -> AV(k) -> Oproj(k)
    Act : q psum->sbuf copy (+bq via activation bias), exp(k), half the
          out biases, wq/wv setup DMA
    Pool: partition_all_reduce (sumexp), A *= 1/sumexp, setup DMA
          (bqt/kmask/yt/wk blocks), last-chunk store fan-out
    DVE : reciprocal, attn psum->sbuf copies, half the out biases
    SP  : x chunk loads, wv/wo loads, output stores

The Qproj(k+1) placement between scores(k) and AV(k) keeps the in-order
PE busy while the softmax chain (Act->Pool->DVE->Pool) for chunk k
drains. A zero-tile matmul warmup burns the initial DMA-latency window
so the PE p-state ramp (0.65->1.2->2.4GHz over ~3us of continuous work)
completes before Qproj(0) issues. wq/wk are host-packed into
per-out-tile [128, contraction] blocks so the first projections start
as soon as the first block lands.
"""

import numpy as np
import ml_dtypes

import concourse.bass as bass
import concourse.bass_isa as bass_isa
import concourse.mybir as mybir
import concourse.tile as tile
from concourse import bacc
from concourse.bass_utils import run_bass_kernel_spmd

F32 = mybir.dt.float32
F32R = mybir.dt.float32r
BF16 = mybir.dt.bfloat16
AF = mybir.ActivationFunctionType

B, T, E, CR, H, D, S = 8, 4096, 640, 768, 8, 80, 77
TC = 512
# (t0, width) chunks; the last 512 is split so the drain phase (softmax ->
# AV -> O -> store with no next-chunk Qproj to overlap) is half as deep.
CHUNKS = [(512 * i, 512) for i in range(7)] + [(3584, 256), (3840, 256)]
NT = len(CHUNKS)
EJ = E // 128            # 5 e-tiles
CJ = CR // 128            # 6 cross-dim tiles
SCALE = float(1.0 / np.sqrt(D))


def _frags():
    fr = []
    for h in range(H):
        e0, e1 = D * h, D * h + D
        for j in range(e0 // 128, (e1 - 1) // 128 + 1):
            p0, p1 = max(0, e0 - 128 * j), min(128, e1 - 128 * j)
            fr.append((h, j, p0, p1))
    return fr


FRAGS = _frags()
NF = len(FRAGS)


def _emit(nc, tc, dr):
    import contextlib

    ctx = contextlib.ExitStack()
    with ctx:
        cpool = ctx.enter_context(tc.tile_pool(name="const", bufs=1))
        pq = ctx.enter_context(tc.tile_pool(name="pq", bufs=2, space="PSUM"))
        psc = ctx.enter_context(tc.tile_pool(name="psc", bufs=2, space="PSUM"))
        pav = ctx.enter_context(tc.tile_pool(name="pav", bufs=2, space="PSUM"))
        pop = ctx.enter_context(tc.tile_pool(name="pop", bufs=2, space="PSUM"))
        xpool = ctx.enter_context(tc.tile_pool(name="xpool", bufs=2))
        qpool = ctx.enter_context(tc.tile_pool(name="qpool", bufs=2))
        apool = ctx.enter_context(tc.tile_pool(name="apool", bufs=2))
        arpool = ctx.enter_context(tc.tile_pool(name="arpool", bufs=3))
        atpool = ctx.enter_context(tc.tile_pool(name="atpool", bufs=2))
        opool = ctx.enter_context(tc.tile_pool(name="opool", bufs=4))

        def load(name, shape, src, dt=F32, eng=None):
            t = cpool.tile(shape, dt, tag=name, name=name)
            (eng or nc.sync).dma_start(t[:], src)
            return t

        def loadw(name, src2, nblk, cols, dt=BF16, eng=None):
            t = cpool.tile([128, nblk, cols], dt, tag=name, name=name)
            (eng or nc.sync).dma_start(
                t[:], src2.rearrange("(b p) c -> p b c", p=128))
            return t

        # ---- load order == DMA order on SP; it gates PE start times.
        # x chunk 0 + per-out-tile wq blocks first so Qproj(0) starts the
        # moment x0 and the first block land; K inputs next (scores(0)),
        # then x1 (Qproj(1)), wv (AV(0)), wo (Oproj(0)).
        def load_x(t0, tw, qtag, engs=None):
            xp = []
            for c in range(EJ):
                xt = xpool.tile([128, TC], BF16, tag=f"xp{c}",
                                name=f"x_{qtag}_{c}")
                eng = engs[c] if engs else nc.sync
                eng.dma_start(xt[0:128, 0:tw],
                              dr["xt"][128 * c:128 * (c + 1), t0:t0 + tw])
                xp.append(xt)
            return xp

        # PE p-state warmup: the clock needs ~3us of continuous work to
        # reach 2.4GHz. Burn the initial DMA-latency window (PE would idle
        # anyway) on dummy matmuls over a zeroed tile so Qproj(0) starts
        # at full clock.
        wz = cpool.tile([128, 128], BF16, tag="wz", name="wz")
        nc.gpsimd.memset(wz[:], 0)
        wp = pav.tile([128, TC], F32, tag="av", name="warm")
        for i in range(16):
            nc.tensor.matmul(wp[:, 0:128], wz[:], wz[:],
                             start=(i == 0), stop=(i == 15))

        # Setup loads fan out over the three DMA-capable engines so no
        # single sequencer serializes the ~8us of weight/x traffic:
        #   SP: x chunks (and all steady-state loads/stores)
        #   Act: wq blocks, then wv
        #   Pool: yt, wk blocks, masks, then wo
        wq_sb = [load(f"wq{j}", [128, E],
                      dr["wq"][128 * j:128 * (j + 1), :], BF16,
                      eng=nc.scalar)
                 for j in range(EJ)]
        # chunk 0's last two x tiles ride Pool's queue so Qproj(0) isn't
        # gated by SP's serial transfer of all five
        xp0 = load_x(0, CHUNKS[0][1], "q0",
                     engs=[nc.sync, nc.sync, nc.sync, nc.gpsimd, nc.gpsimd])
        bqt = load("bqt", [128, EJ], dr["bqt"][:], eng=nc.gpsimd)
        kmask = load("kmask", [128, NF], dr["kmask"][:], eng=nc.gpsimd)
        ytp_t = loadw("yt", dr["yt"][:], CJ, S, eng=nc.gpsimd)
        wkb = [load(f"wkb{j}", [128, CR],
                    dr["wk"][128 * j:128 * (j + 1), :], BF16,
                    eng=nc.gpsimd)
               for j in range(EJ)]
        xp1 = load_x(TC, TC, "q1")
        wv_t = loadw("wv", dr["wv"][:], CJ, E)
        vmask = load("vmask", [S, 128 * NF], dr["vmask"][:])
        wo_t = loadw("wo", dr["wo"][:], EJ, E)
        bot = load("bot", [128, EJ], dr["bot"][:])
        wv_sb = [wv_t[:, c, :] for c in range(CJ)]
        wo_sb = [wo_t[:, c, :] for c in range(EJ)]
        ytp = [ytp_t[:, c, :] for c in range(CJ)]

        def emit_qproj(xp, tw, qtag):
            qs = []
            for j in range(EJ):
                qp = pq.tile([128, TC], F32, tag="qp", name=f"qp{j}")
                for c in range(EJ):
                    nc.tensor.matmul(qp[0:128, 0:tw],
                                     wq_sb[j][:, 128 * c:128 * (c + 1)],
                                     xp[c][0:128, 0:tw],
                                     start=(c == 0), stop=(c == EJ - 1))
                q = qpool.tile([128, TC], BF16, tag=f"q{j}", name=f"{qtag}_{j}")
                with nc.allow_low_precision(reason="bf16 attention"):
                    nc.scalar.activation(q[0:128, 0:tw], qp[0:128, 0:tw],
                                         AF.Identity, bias=bqt[:, j:j + 1])
                qs.append(q)
            return qs

        qs_cur = emit_qproj(xp0, TC, "q0")

        # ---- K projection -> per-fragment zero-masked staging (bf16) ----
        kstage = [cpool.tile([128, S], BF16, tag=f"ks{fi}", name=f"ks{fi}")
                  for fi in range(NF)]
        for j in range(EJ):
            kp = pop.tile([128, S], F32, tag="op")
            for c in range(CJ):
                nc.tensor.matmul(kp[:], wkb[j][:, 128 * c:128 * (c + 1)],
                                 ytp[c], start=(c == 0), stop=(c == CJ - 1))
            for fi, (h, jj, p0, p1) in enumerate(FRAGS):
                if jj != j:
                    continue
                with nc.allow_low_precision(reason="bf16 attention"):
                    if fi % 2 == 0:
                        nc.vector.tensor_scalar_mul(
                            kstage[fi][:], kp[:], kmask[:, fi:fi + 1])
                    else:
                        nc.scalar.activation(
                            kstage[fi][:], kp[:], AF.Identity,
                            scale=kmask[:, fi:fi + 1])

        vb = [cpool.tile([S, 128], BF16, tag=f"vb{fi}", name=f"vb{fi}")
              for fi in range(NF)]

        def emit_vproj():
            # V projection -> per-fragment zero-masked vb tiles (bf16)
            for (n0, n1) in ((0, 512), (512, E)):
                vp = pav.tile([S, n1 - n0], F32, tag="av")
                for c in range(CJ):
                    nc.tensor.matmul(vp[:], ytp[c], wv_sb[c][:, n0:n1],
                                     start=(c == 0), stop=(c == CJ - 1))
                for fi, (h, j, p0, p1) in enumerate(FRAGS):
                    c0 = 128 * j
                    if not (n0 <= c0 and c0 + 128 <= n1):
                        continue
                    with nc.allow_low_precision(reason="bf16 attention"):
                        nc.vector.tensor_mul(vb[fi][:],
                                             vp[:, c0 - n0:c0 - n0 + 128],
                                             vmask[:, 128 * fi:128 * (fi + 1)])

        # ---- main loop over token chunks (software-pipelined) ----
        xp_next = xp1
        for it in range(NT):
            t0, tw = CHUNKS[it]
            # prefetch x for chunk it+1 (SP); chunk 1 was hoisted above
            if it >= 1 and it + 1 < NT:
                nt0, ntw = CHUNKS[it + 1]
                xp_next = load_x(nt0, ntw, f"q{it + 1}")

            # scores -> exp -> sumexp(all-reduce) -> recip -> normalize
            aps = []
            for h in range(H):
                frs = [(fi, f) for fi, f in enumerate(FRAGS) if f[0] == h]
                sc = psc.tile([S, TC], F32, tag="sc")
                for i, (fi, (hh, j, p0, p1)) in enumerate(frs):
                    nc.tensor.matmul(sc[0:S, 0:tw], kstage[fi][:],
                                     qs_cur[j][0:128, 0:tw],
                                     start=(i == 0), stop=(i == len(frs) - 1))
                a = apool.tile([S, TC], BF16, tag=f"a{h}")
                with nc.allow_low_precision(reason="bf16 attention"):
                    nc.scalar.activation(a[0:S, 0:tw], sc[0:S, 0:tw],
                                         AF.Exp, scale=SCALE)
                # f32 sumexp so reciprocal_approx_fast (single custom DVE
                # op, ~5x cheaper than bit-exact reciprocal on HW) applies
                ar = arpool.tile([S, TC], F32, tag="ar")
                with nc.allow_low_precision(reason="bf16 attention"):
                    nc.gpsimd.partition_all_reduce(
                        ar[0:S, 0:tw], a[0:S, 0:tw], channels=S,
                        reduce_op=bass_isa.ReduceOp.add)
                    nc.vector.reciprocal_approx_fast(
                        out=ar[0:S, 0:tw], in_=ar[0:S, 0:tw])
                    nc.gpsimd.tensor_mul(a[0:S, 0:tw], a[0:S, 0:tw],
                                         ar[0:S, 0:tw])
                aps.append(a)

            # Qproj for the NEXT chunk fills PE while softmax drains
            if it + 1 < NT:
                qs_cur = emit_qproj(xp_next, CHUNKS[it + 1][1], f"q{it + 1}")
            if it == 0:
                emit_vproj()

            # AV: accumulate head fragments into attn' tiles (bf16)
            attn = []
            for j in range(EJ):
                av = pav.tile([128, TC], F32, tag="av")
                frs = [(fi, f) for fi, f in enumerate(FRAGS) if f[1] == j]
                for i, (fi, (h, jj, p0, p1)) in enumerate(frs):
                    nc.tensor.matmul(av[0:128, 0:tw], vb[fi][:],
                                     aps[h][0:S, 0:tw],
                                     start=(i == 0), stop=(i == len(frs) - 1))
                at = atpool.tile([128, TC], BF16, tag=f"at{j}")
                with nc.allow_low_precision(reason="bf16 attention"):
                    nc.vector.tensor_copy(at[0:128, 0:tw], av[0:128, 0:tw])
                attn.append(at)

            # O projection + bias (f32 out), store via SP
            for p in range(EJ):
                op = pop.tile([128, TC], F32, tag="op")
                for j in range(EJ):
                    nc.tensor.matmul(op[0:128, 0:tw],
                                     wo_sb[j][:, 128 * p:128 * (p + 1)],
                                     attn[j][0:128, 0:tw],
                                     start=(j == 0), stop=(j == EJ - 1))
                ob = opool.tile([128, TC], F32, tag="ob")
                if it == NT - 1 and p == EJ - 1:
                    # final tile of the run: bias the two column halves on
                    # DVE and Act concurrently so the last store issues
                    # as early as possible
                    hw2 = tw // 2
                    nc.vector.tensor_scalar_add(ob[0:128, 0:hw2],
                                                op[0:128, 0:hw2],
                                                bot[:, p:p + 1])
                    nc.scalar.activation(ob[0:128, hw2:tw],
                                         op[0:128, hw2:tw],
                                         AF.Identity, bias=bot[:, p:p + 1])
                elif p % 2 == 0 and it != NT - 2:
                    nc.vector.tensor_scalar_add(ob[0:128, 0:tw],
                                                op[0:128, 0:tw],
                                                bot[:, p:p + 1])
                else:
                    nc.scalar.activation(ob[0:128, 0:tw], op[0:128, 0:tw],
                                         AF.Identity, bias=bot[:, p:p + 1])
                # drain stores across SP/Act/Pool on the last chunk so the
                # final writeback isn't serialized on one engine
                if it < NT - 2:
                    seng = nc.sync
                elif it == NT - 2:
                    seng = (nc.sync, nc.gpsimd)[p % 2]
                else:
                    seng = (nc.sync, nc.gpsimd, nc.sync, nc.gpsimd,
                            nc.scalar)[p]
                if it == NT - 1 and p == EJ - 1:
                    hw2 = tw // 2
                    nc.sync.dma_start(
                        dr["ot"][128 * p:128 * (p + 1), t0:t0 + hw2],
                        ob[0:128, 0:hw2])
                    nc.gpsimd.dma_start(
                        dr["ot"][128 * p:128 * (p + 1), t0 + hw2:t0 + tw],
                        ob[0:128, hw2:tw])
                else:
                    seng.dma_start(
                        dr["ot"][128 * p:128 * (p + 1), t0:t0 + tw],
                        ob[0:128, 0:tw])


def build_program():
    nc = bacc.Bacc("TRN2", target_bir_lowering=False, debug=False,
                   num_devices=B)
    dr = {}

    def din(name, shape, dt=F32):
        dr[name] = nc.dram_tensor(name, shape, dt, kind="ExternalInput")
        return dr[name]

    din("xt", [E, T], BF16)
    din("yt", [CR, S], BF16)
    din("wq", [E, E], BF16)
    din("wk", [E, CR], BF16)
    din("wv", [CR, E], BF16)
    din("wo", [E, E], BF16)
    din("bqt", [128, EJ])
    din("bot", [128, EJ])
    din("kmask", [128, NF])
    din("vmask", [S, 128 * NF])
    dr["ot"] = nc.dram_tensor("ot", [E, T], F32, kind="ExternalOutput")

    with tile.TileContext(nc) as tc:
        _emit(nc, tc, {k: v[:] for k, v in dr.items()})
    nc.compile()
    return nc


def make_in_maps(x, y, Wq, bq, Wk, bk, Wv, bv, Wo, bo):
    bf = lambda a: np.ascontiguousarray(
        np.asarray(a, dtype=np.float32).astype(ml_dtypes.bfloat16))
    f = lambda a: np.ascontiguousarray(np.asarray(a, dtype=np.float32))
    kmask = np.zeros((128, NF), np.float32)
    vmask = np.zeros((S, 128 * NF), np.float32)
    for fi, (h, j, p0, p1) in enumerate(FRAGS):
        kmask[p0:p1, fi] = 1.0
        vmask[:, 128 * fi + p0:128 * fi + p1] = 1.0
    # exact folds: k bias cancels in softmax; v bias folds into bo.
    bo_eff = (np.asarray(bo, np.float32)
              + np.asarray(bv, np.float32) @ np.asarray(Wo, np.float32))
    # wq packed per out-col-block j: wqp[j, p, 128*b + c] = Wq[128*b + p,
    # 128*j + c] so Qproj out-tile j loads one contiguous [128, E] block.
    wqp = (np.asarray(Wq, np.float32)
           .reshape(EJ, 128, EJ, 128).transpose(2, 1, 0, 3)
           .reshape(E, E))
    # wk packed the same way: block j holds Wk[:, j-cols] as [128, CR]
    wkp = (np.asarray(Wk, np.float32)
           .reshape(CJ, 128, EJ, 128).transpose(2, 1, 0, 3)
           .reshape(E, CR))
    shared = dict(
        wq=bf(wqp), wk=bf(wkp), wv=bf(Wv), wo=bf(Wo),
        bqt=f(np.asarray(bq, np.float32).reshape(EJ, 128).T),
        bot=f(bo_eff.reshape(EJ, 128).T),
        kmask=kmask,
        vmask=vmask,
    )
    x = np.asarray(x, np.float32)
    y = np.asarray(y, np.float32)
    in_maps = []
    for b in range(B):
        m = dict(shared)
        m["xt"] = bf(x[b].T)
        m["yt"] = bf(y[b].T)
        in_maps.append(m)
    return in_maps


def assemble_output(results):
    return np.stack([results[b]["ot"].T for b in range(B)], axis=0)


_PROG = None


def _prog():
    global _PROG
    if _PROG is None:
        _PROG = build_program()
    return _PROG


def kernel(x, y, Wq, bq, Wk, bk, Wv, bv, Wo, bo):
    nc = _prog()
    in_maps = make_in_maps(x, y, Wq, bq, Wk, bk, Wv, bv, Wo, bo)
    res = run_bass_kernel_spmd(nc, in_maps, core_ids=list(range(B)))
    return assemble_output(res.results)
